# revision 10
# baseline (speedup 1.0000x reference)
"""MultiHeadAttention (no-transpose head reshape) on 8 TRN2 NeuronCores.

The reference reshapes [B,S,D] -> [B,H,S',dk] WITHOUT transposing, so
"head h" of batch b is exactly rows [128h, 128h+128) of x viewed as 2048
pseudo-tokens of dim 64: pseudo-token (r, c) of head-block t is
x[t*128+r] features [c*64, c*64+64).  The whole problem is data-parallel
over the 32 (b,h) pairs: each of 8 cores owns 4 head-blocks (512 rows) of
one batch, no communication needed.

Internally pseudo-tokens are enumerated C-MAJOR (k'' = c*128 + r), which
is legal because softmax just sums over all keys (any consistent
permutation of keys works, and the query permutation is undone in the
final reorg).  With that ordering the "V with ones column" chunks are
natural slices of V, and every reorg copy moves contiguous 128-element
runs.

Per-core pipeline (bf16 matmuls, f32 accumulation):
  1. x -> bf16 -> xT via PE transposes (warms up the PE).
  2. V = x@Wv+bv straight into the ones-padded Vno layout;
     QT/KT = (x@W)^T with weight chunks stationary.
  3. QhT/KhT per-head [64,2048] via contiguous DVE copies.
  4. Attention per head: scoresT[k,q] = KhT^T@QhT (PSUM) -> exp on ACT
     (1/8 scale fused) -> probsT bf16 -> ctxT[65,q] += Vno^T@probsT,
     row 64 of ctxT accumulates softmax denominators (ones column).
  5. Normalize (DVE divide) + reorg into cbT (out-proj lhsT layout).
  6. out = cb@Wo + bo.

PE accumulation chains are emitted pairwise-interleaved so consecutive
matmuls hit different PSUM banks (fill overlaps drain).
"""

import sys

if "/opt/trn_rl_repo" not in sys.path:
    sys.path.insert(0, "/opt/trn_rl_repo")

import numpy as np

import concourse.bacc as bacc
import concourse.mybir as mybir
import concourse.tile as tile
from concourse.bass_utils import run_bass_kernel_spmd
from concourse.masks import make_identity

F32 = mybir.dt.float32
BF16 = mybir.dt.bfloat16
AF = mybir.ActivationFunctionType
ALU = mybir.AluOpType

N_CORES = 8
D = 1024
ROWS = 512          # rows of x per core
T = 4               # head-blocks (= heads) per core
NJ = 8              # 128-feature chunks of D
DK = 64
S2 = 2048           # pseudo-sequence length per head
GROUPS = (3, 3, 3, 3, 2, 2)   # k-chunks per exp group (sums to 16)
DEBUG = False


def _interleave(*seqs):
    """Round-robin the callables in seqs (lists of thunks), call in order."""
    n = max(len(s) for s in seqs)
    for u in range(n):
        for s in seqs:
            if u < len(s):
                s[u]()


def _emit(nc, tc, pools):
    persist = pools["persist"]
    stage = pools["stage"]
    psum_s = pools["psum_s"]      # tag "s": [128,3,512] f32, bufs=2 (6 banks)
    psum_ctx = pools["psum_ctx"]  # tag "ctx": [65,512] f32, bufs=2 (2 banks)

    x_d = nc.dram_tensor("x", [ROWS, D], F32, kind="ExternalInput")
    w_d = {}
    b_d = {}
    for w in ("q", "k", "v", "o"):
        w_d[w] = nc.dram_tensor(f"W{w}", [D, D], F32, kind="ExternalInput")
        b_d[w] = nc.dram_tensor(f"b{w}", [D], F32, kind="ExternalInput")
    out_d = nc.dram_tensor("out", [ROWS, D], F32, kind="ExternalOutput")

    # ---- persistent SBUF tensors ----
    xT = persist.tile([128, NJ, ROWS], BF16, name="xT")
    Wbf = {w: persist.tile([128, NJ, D], BF16, name=f"W{w}bf") for w in ("v", "q", "k", "o")}
    QT = persist.tile([128, NJ, ROWS], BF16, name="QT")
    KT = persist.tile([128, NJ, ROWS], BF16, name="KT")
    Vno = persist.tile([128, T, 16, DK + 1], BF16, name="Vno")
    QhT = persist.tile([128, 2, S2], BF16, name="QhT")
    KhT = persist.tile([128, 2, S2], BF16, name="KhT")
    cbT = persist.tile([128, T, NJ, 128], BF16, name="cbT")
    bq_sb = persist.tile([128, NJ], F32, name="bq_sb")
    bk_sb = persist.tile([128, NJ], F32, name="bk_sb")
    bvB = persist.tile([128, D], F32, name="bvB")
    boB = persist.tile([128, D], F32, name="boB")
    ident = persist.tile([128, 128], BF16, name="ident")

    make_identity(nc, ident[:])

    # pre-warm the exp table-set while ACT is otherwise idle
    dummy = persist.tile([1, 8], F32, name="dummy")
    nc.vector.memset(dummy[:], 0.0)
    nc.scalar.activation(dummy[:], dummy[:], AF.Exp, scale=1.0)

    # ---- biases (small, early) ----
    nc.sync.dma_start(out=bq_sb[:], in_=b_d["q"].ap().rearrange("(j p) -> p j", p=128))
    nc.sync.dma_start(out=bk_sb[:], in_=b_d["k"].ap().rearrange("(j p) -> p j", p=128))
    nc.sync.dma_start(out=bvB[0:1, :], in_=b_d["v"].ap().unsqueeze(0))
    nc.gpsimd.partition_broadcast(bvB[:], bvB[0:1, :])
    nc.sync.dma_start(out=boB[0:1, :], in_=b_d["o"].ap().unsqueeze(0))
    nc.gpsimd.partition_broadcast(boB[:], boB[0:1, :])

    # ones columns of Vno (overwritten below except column 64)
    nc.gpsimd.memset(Vno[:], 1.0)

    # ---- stage 0: x -> xT (PE transposes; also warms up the PE) ----
    for t in range(T):
        xs = stage.tile([128, D], F32, tag="xstage")
        nc.sync.dma_start(out=xs[:], in_=x_d[t * 128:(t + 1) * 128, :])
        xb = stage.tile([128, D], BF16, tag="xbf")
        nc.vector.tensor_copy(out=xb[:], in_=xs[:])
        for i in range(NJ):
            tp = psum_s.tile([128, 128], BF16, tag="s", name="tp")
            nc.tensor.transpose(tp[:], xb[:, i * 128:(i + 1) * 128], ident[:])
            nc.vector.tensor_copy(out=xT[:, i, t * 128:(t + 1) * 128], in_=tp[:])

    # ---- weights: DMA + DVE cast, in consumption order ----
    for w in ("v", "q", "k", "o"):
        for i in range(NJ):
            ws = stage.tile([128, D], F32, tag="wstage")
            nc.sync.dma_start(out=ws[:], in_=w_d[w][i * 128:(i + 1) * 128, :])
            nc.vector.tensor_copy(out=Wbf[w][:, i, :], in_=ws[:])

    # ---- stage 1: projections (chains emitted pairwise for PE overlap) ----
    def v_chain(t, h):
        ps = psum_s.tile([128, 3, 512], F32, tag="s", name="psv")

        def mm(i):
            return lambda: nc.tensor.matmul(
                ps[:, 0, :],
                lhsT=xT[:, i, t * 128:(t + 1) * 128],
                rhs=Wbf["v"][:, i, h * 512:(h + 1) * 512],
                start=(i == 0),
                stop=(i == NJ - 1),
            )

        def evict():
            nc.vector.tensor_tensor(
                out=Vno[:, t, h * 8:(h + 1) * 8, 0:DK],
                in0=ps[:, 0, :].rearrange("p (c d) -> p c d", c=8),
                in1=bvB[:, h * 512:(h + 1) * 512].rearrange("p (c d) -> p c d", c=8),
                op=ALU.add,
            )

        return [mm(i) for i in range(NJ)], evict

    def qk_chain(wname, bsb, dst, j):
        ps = psum_s.tile([128, 3, 512], F32, tag="s", name="psqk")

        def mm(i):
            return lambda: nc.tensor.matmul(
                ps[:, 0, :],
                lhsT=Wbf[wname][:, i, j * 128:(j + 1) * 128],
                rhs=xT[:, i, :],
                start=(i == 0),
                stop=(i == NJ - 1),
            )

        def evict():
            nc.vector.tensor_scalar(
                out=dst[:, j, :], in0=ps[:, 0, :],
                scalar1=bsb[:, j:j + 1], scalar2=None, op0=ALU.add,
            )

        return [mm(i) for i in range(NJ)], evict

    chains = [v_chain(0, 0), v_chain(0, 1)]
    for j in range(NJ):
        chains.append(qk_chain("q", bq_sb, QT, j))
        chains.append(qk_chain("k", bk_sb, KT, j))
    for t in range(1, T):
        for h in range(2):
            chains.append(v_chain(t, h))
    for a in range(0, len(chains), 2):
        mmsA, evA = chains[a]
        mmsB, evB = chains[a + 1]
        _interleave(mmsA, mmsB)
        evA()
        evB()

    # ---- stage 2: QhT/KhT reorg (contiguous DVE copies) ----
    # QhT[eh*64+d', p, c*128+r] = QT[(c%2)*64+d', c//2, t*128+r],  t=2p+eh
    # t=0 is copied per-j so attention can start as soon as the first
    # projection chains finish; later heads are copied coarsely.
    for j in range(NJ):
        for srcT, dsthT in ((QT, QhT), (KT, KhT)):
            for ce in range(2):
                c = 2 * j + ce
                nc.vector.tensor_copy(
                    out=dsthT[0:64, 0, c * 128:(c + 1) * 128],
                    in_=srcT[ce * 64:(ce + 1) * 64, j, 0:128],
                )
    for srcT, dsthT in ((QT, QhT), (KT, KhT)):
        for t in range(1, T):
            p, eh = t // 2, t % 2
            for ce in range(2):
                src = srcT[ce * 64:(ce + 1) * 64, :, t * 128:(t + 1) * 128]
                dst = dsthT[eh * 64:(eh + 1) * 64, p, :].rearrange(
                    "p (c r) -> p c r", c=16
                )[:, ce::2, :]
                nc.vector.tensor_copy(out=dst, in_=src)

    # ---- stage 3: attention (software-pipelined scores/ctx interleave) ----
    if DEBUG:
        dbg_ctx = nc.dram_tensor("dbg_ctx", [4, DK + 1, 512], F32, kind="ExternalOutput")
        dbg_pr = nc.dram_tensor("dbg_pr", [128, 3, 512], F32, kind="ExternalOutput")
        dbg_s = nc.dram_tensor("dbg_s", [128, 3, 512], F32, kind="ExternalOutput")
        dbg_rsum = nc.dram_tensor("dbg_rsum", [4, 64, 512], F32, kind="ExternalOutput")
    for t in range(T):
        p, eh = t // 2, t % 2
        qh = QhT[eh * 64:(eh + 1) * 64, p, :]
        kh = KhT[eh * 64:(eh + 1) * 64, p, :]
        for qq in range(4):
            ctx = psum_ctx.tile([DK + 1, 512], F32, tag="ctx")
            pend_ctx = []  # ctx-matmul thunks of the previous group
            kc0 = 0
            for gi, gsz in enumerate(GROUPS):
                s = psum_s.tile([128, 3, 512], F32, tag="s", name="satt")
                sco = []
                for u in range(gsz):
                    kc = kc0 + u

                    def mk_s(u=u, kc=kc, s=s):
                        nc.tensor.matmul(
                            s[:, u, :],
                            lhsT=kh[:, kc * 128:(kc + 1) * 128],
                            rhs=qh[:, qq * 512:(qq + 1) * 512],
                            start=True,
                            stop=True,
                        )

                    sco.append(mk_s)
                _interleave(sco, pend_ctx)
                if DEBUG and t == 0 and qq == 0 and gi == 0:
                    sf = stage.tile([128, 3, 512], F32, tag="prf")
                    nc.vector.tensor_copy(out=sf[:], in_=s[:])
                    nc.scalar.dma_start(out=dbg_s[:], in_=sf[:])
                pr = stage.tile([128, 3, 512], BF16, tag="pr", bufs=3)
                nc.scalar.activation(
                    pr[:, 0:gsz, :], s[:, 0:gsz, :], AF.Exp, scale=0.125
                )
                if DEBUG and t == 0 and qq == 0 and gi == 0:
                    prf = stage.tile([128, 3, 512], F32, tag="prf")
                    nc.vector.tensor_copy(out=prf[:], in_=pr[:])
                    nc.scalar.dma_start(out=dbg_pr[:], in_=prf[:])
                pend_ctx = []
                for u in range(gsz):
                    kc = kc0 + u

                    def mk_c(u=u, kc=kc, pr=pr):
                        nc.tensor.matmul(
                            ctx[:],
                            lhsT=Vno[:, t, kc, :],
                            rhs=pr[:, u, :],
                            start=(kc == 0),
                            stop=(kc == 15),
                        )

                    pend_ctx.append(mk_c)
                kc0 += gsz
            for c in pend_ctx:
                c()
            if DEBUG and t == 0:
                cf = stage.tile([DK + 1, 512], F32, tag="ctxf")
                nc.vector.tensor_copy(out=cf[:], in_=ctx[:])
                nc.scalar.dma_start(out=dbg_ctx[qq, :, :], in_=cf[:])

            # normalize by softmax sums (ctx row 64) and reorg into cbT:
            # cbT[(c%2)*64+d', t, c//2, r] = ctx[d', (c-4qq)*128+r] / sums
            sums = stage.tile([1, 512], F32, tag="sums")
            nc.vector.tensor_copy(out=sums[:], in_=ctx[DK:DK + 1, :])
            rsum = stage.tile([1, 512], F32, tag="rsum")
            nc.vector.reciprocal_approx_fast(out=rsum[:], in_=sums[:])
            rsumB = stage.tile([64, 512], F32, tag="rsumB")
            nc.gpsimd.partition_broadcast(rsumB[:], rsum[:])
            if DEBUG and t == 0:
                nc.scalar.dma_start(out=dbg_rsum[qq, :, :], in_=rsumB[:])
            for ce in range(2):
                csrc = ctx[0:64, :].rearrange("p (c r) -> p c r", c=4)[:, ce::2, :]
                sc = rsumB[:].rearrange("p (c r) -> p c r", c=4)[:, ce::2, :]
                dst = cbT[ce * 64:(ce + 1) * 64, t, 2 * qq:2 * qq + 2, :]
                nc.vector.tensor_tensor(out=dst, in0=csrc, in1=sc, op=ALU.mult)

    # ---- stage 4: output projection (pairwise-interleaved chains) ----
    def o_chain(t, h):
        ps = psum_s.tile([128, 3, 512], F32, tag="s", name="pso")

        def mm(j):
            return lambda: nc.tensor.matmul(
                ps[:, 0, :],
                lhsT=cbT[:, t, j, :],
                rhs=Wbf["o"][:, j, h * 512:(h + 1) * 512],
                start=(j == 0),
                stop=(j == NJ - 1),
            )

        def evict():
            osb = stage.tile([128, 512], F32, tag="ostage")
            nc.vector.tensor_tensor(
                out=osb[:], in0=ps[:, 0, :], in1=boB[:, h * 512:(h + 1) * 512],
                op=ALU.add,
            )
            nc.scalar.dma_start(
                out=out_d[t * 128:(t + 1) * 128, h * 512:(h + 1) * 512], in_=osb[:]
            )

        return [mm(j) for j in range(NJ)], evict

    ochains = [o_chain(t, h) for t in range(T) for h in range(2)]
    for a in range(0, len(ochains), 2):
        mmsA, evA = ochains[a]
        mmsB, evB = ochains[a + 1]
        _interleave(mmsA, mmsB)
        evA()
        evB()

    if DEBUG:
        for nm, sb in (("dbg_xT", xT), ("dbg_QT", QT), ("dbg_KT", KT),
                       ("dbg_Vno", Vno), ("dbg_QhT", QhT), ("dbg_KhT", KhT),
                       ("dbg_cbT", cbT)):
            dd = nc.dram_tensor(nm, list(sb.shape), BF16, kind="ExternalOutput")
            nc.scalar.dma_start(out=dd[:], in_=sb[:])


_CACHE = {}


def build():
    if "nc" in _CACHE:
        return _CACHE["nc"]
    nc = bacc.Bacc(None, target_bir_lowering=False)
    with tile.TileContext(nc) as tc:
        import contextlib

        with contextlib.ExitStack() as ctx:
            pools = {
                "persist": ctx.enter_context(tc.tile_pool(name="persist", bufs=1)),
                "stage": ctx.enter_context(tc.tile_pool(name="stage", bufs=2)),
                "psum_s": ctx.enter_context(
                    tc.tile_pool(name="psum_s", bufs=2, space="PSUM")
                ),
                "psum_ctx": ctx.enter_context(
                    tc.tile_pool(name="psum_ctx", bufs=2, space="PSUM")
                ),
            }
            _emit(nc, tc, pools)
    nc.compile()
    _CACHE["nc"] = nc
    return nc


def kernel(x, Wq, bq, Wk, bk, Wv, bv, Wo, bo, _trace=False, _tmpdir=None):
    x = np.ascontiguousarray(np.asarray(x, dtype=np.float32))
    full = {
        "Wq": np.ascontiguousarray(np.asarray(Wq, np.float32)),
        "bq": np.ascontiguousarray(np.asarray(bq, np.float32)),
        "Wk": np.ascontiguousarray(np.asarray(Wk, np.float32)),
        "bk": np.ascontiguousarray(np.asarray(bk, np.float32)),
        "Wv": np.ascontiguousarray(np.asarray(Wv, np.float32)),
        "bv": np.ascontiguousarray(np.asarray(bv, np.float32)),
        "Wo": np.ascontiguousarray(np.asarray(Wo, np.float32)),
        "bo": np.ascontiguousarray(np.asarray(bo, np.float32)),
    }
    B, S, Dm = x.shape
    assert (B, S, Dm) == (2, 2048, 1024), (B, S, Dm)

    nc = build()
    in_maps = []
    for core in range(N_CORES):
        b, blk = core // 4, core % 4
        m = dict(full)
        m["x"] = np.ascontiguousarray(x[b, blk * ROWS:(blk + 1) * ROWS, :])
        in_maps.append(m)

    res = run_bass_kernel_spmd(
        nc,
        in_maps,
        core_ids=list(range(N_CORES)),
        trace=_trace,
        tmpdir=_tmpdir,
    )
    out = np.empty((B, S, Dm), np.float32)
    for core in range(N_CORES):
        b, blk = core // 4, core % 4
        out[b, blk * ROWS:(blk + 1) * ROWS, :] = res.results[core]["out"]
    if _trace:
        return out, res
    return out


# revision 12
# speedup vs baseline: 1.0391x; 1.0391x over previous
"""MultiHeadAttention (no-transpose head reshape) on 8 TRN2 NeuronCores.

The reference reshapes [B,S,D] -> [B,H,S',dk] WITHOUT transposing, so
"head h" of batch b is exactly rows [128h, 128h+128) of x viewed as 2048
pseudo-tokens of dim 64: pseudo-token (r, c) of head-block t is
x[t*128+r] features [c*64, c*64+64).  The whole problem is data-parallel
over the 32 (b,h) pairs: each of 8 cores owns 4 head-blocks (512 rows) of
one batch, no communication needed.

Internally pseudo-tokens are enumerated C-MAJOR (k'' = c*128 + r), which
is legal because softmax just sums over all keys (any consistent
permutation of keys works, and the query permutation is undone in the
final reorg).  With that ordering the "V with ones column" chunks are
natural slices of V, and every reorg copy moves contiguous 128-element
runs.

Per-core pipeline (bf16 matmuls, f32 accumulation):
  1. x -> bf16 -> xT via PE transposes (warms up the PE).
  2. V = x@Wv+bv straight into the ones-padded Vno layout;
     QT/KT = (x@W)^T with weight chunks stationary.
  3. QhT/KhT per-head [64,2048] via contiguous DVE copies.
  4. Attention per head: scoresT[k,q] = KhT^T@QhT (PSUM) -> exp on ACT
     (1/8 scale fused) -> probsT bf16 -> ctxT[65,q] += Vno^T@probsT,
     row 64 of ctxT accumulates softmax denominators (ones column).
  5. Normalize (DVE divide) + reorg into cbT (out-proj lhsT layout).
  6. out = cb@Wo + bo.

PE accumulation chains are emitted pairwise-interleaved so consecutive
matmuls hit different PSUM banks (fill overlaps drain).
"""

import sys

if "/opt/trn_rl_repo" not in sys.path:
    sys.path.insert(0, "/opt/trn_rl_repo")

import numpy as np

import concourse.bacc as bacc
import concourse.mybir as mybir
import concourse.tile as tile
from concourse.bass_utils import run_bass_kernel_spmd
from concourse.masks import make_identity

F32 = mybir.dt.float32
BF16 = mybir.dt.bfloat16
AF = mybir.ActivationFunctionType
ALU = mybir.AluOpType

N_CORES = 8
D = 1024
ROWS = 512          # rows of x per core
T = 4               # head-blocks (= heads) per core
NJ = 8              # 128-feature chunks of D
DK = 64
S2 = 2048           # pseudo-sequence length per head
GROUPS = (3, 3, 3, 3, 2, 2)   # k-chunks per exp group (sums to 16)
DEBUG = False


def _interleave(*seqs):
    """Round-robin the callables in seqs (lists of thunks), call in order."""
    n = max(len(s) for s in seqs)
    for u in range(n):
        for s in seqs:
            if u < len(s):
                s[u]()


def _emit(nc, tc, pools):
    persist = pools["persist"]
    stage = pools["stage"]
    psum_s = pools["psum_s"]      # tag "s": [128,3,512] f32, bufs=2 (6 banks)
    psum_ctx = pools["psum_ctx"]  # tag "ctx": [65,512] f32, bufs=2 (2 banks)

    x_d = nc.dram_tensor("x", [ROWS, D], F32, kind="ExternalInput")
    w_d = {}
    b_d = {}
    for w in ("q", "k", "v", "o"):
        w_d[w] = nc.dram_tensor(f"W{w}", [D, D], F32, kind="ExternalInput")
        b_d[w] = nc.dram_tensor(f"b{w}", [D], F32, kind="ExternalInput")
    out_d = nc.dram_tensor("out", [ROWS, D], F32, kind="ExternalOutput")

    # ---- persistent SBUF tensors ----
    xT = persist.tile([128, NJ, ROWS], BF16, name="xT")
    Wbf = {w: persist.tile([128, NJ, D], BF16, name=f"W{w}bf") for w in ("v", "q", "k", "o")}
    QT = persist.tile([128, NJ, ROWS], BF16, name="QT")
    KT = persist.tile([128, NJ, ROWS], BF16, name="KT")
    Vno = persist.tile([128, T, 16, DK + 1], BF16, name="Vno")
    QhF = persist.tile([128, T, S2], BF16, name="QhF")
    KhF = persist.tile([128, T, S2], BF16, name="KhF")
    cbT = persist.tile([128, T, NJ, 128], BF16, name="cbT")
    bq_sb = persist.tile([128, NJ], F32, name="bq_sb")
    bk_sb = persist.tile([128, NJ], F32, name="bk_sb")
    bvB = persist.tile([128, D], F32, name="bvB")
    boB = persist.tile([128, D], F32, name="boB")
    ident = persist.tile([128, 128], BF16, name="ident")

    make_identity(nc, ident[:])

    # pre-warm the exp table-set while ACT is otherwise idle
    dummy = persist.tile([1, 8], F32, name="dummy")
    nc.vector.memset(dummy[:], 0.0)
    nc.scalar.activation(dummy[:], dummy[:], AF.Exp, scale=1.0)

    # ---- biases (small, early) ----
    nc.sync.dma_start(out=bq_sb[:], in_=b_d["q"].ap().rearrange("(j p) -> p j", p=128))
    nc.sync.dma_start(out=bk_sb[:], in_=b_d["k"].ap().rearrange("(j p) -> p j", p=128))
    nc.sync.dma_start(out=bvB[0:1, :], in_=b_d["v"].ap().unsqueeze(0))
    nc.gpsimd.partition_broadcast(bvB[:], bvB[0:1, :])
    nc.sync.dma_start(out=boB[0:1, :], in_=b_d["o"].ap().unsqueeze(0))
    nc.gpsimd.partition_broadcast(boB[:], boB[0:1, :])

    # ones columns of Vno (overwritten below except column 64)
    nc.gpsimd.memset(Vno[:], 1.0)

    # ---- stage 0: x -> xT (PE transposes; also warms up the PE) ----
    for t in range(T):
        xs = stage.tile([128, D], F32, tag="xstage")
        nc.sync.dma_start(out=xs[:], in_=x_d[t * 128:(t + 1) * 128, :])
        xb = stage.tile([128, D], BF16, tag="xbf")
        nc.vector.tensor_copy(out=xb[:], in_=xs[:])
        for i in range(NJ):
            tp = psum_s.tile([128, 128], BF16, tag="s", name="tp")
            nc.tensor.transpose(tp[:], xb[:, i * 128:(i + 1) * 128], ident[:])
            nc.scalar.copy(out=xT[:, i, t * 128:(t + 1) * 128], in_=tp[:])

    # ---- weights: DMA + cast, in consumption order (Wo on gpsimd: late) ----
    for w in ("v", "q", "k", "o"):
        for i in range(NJ):
            ws = stage.tile([128, D], F32, tag="wstage")
            nc.sync.dma_start(out=ws[:], in_=w_d[w][i * 128:(i + 1) * 128, :])
            if w == "o":
                nc.gpsimd.tensor_copy(out=Wbf[w][:, i, :], in_=ws[:])
            else:
                nc.vector.tensor_copy(out=Wbf[w][:, i, :], in_=ws[:])

    # ---- stage 1: projections (chains emitted pairwise for PE overlap) ----
    def v_chain(t, h):
        ps = psum_s.tile([128, 3, 512], F32, tag="s", name="psv")

        def mm(i):
            return lambda: nc.tensor.matmul(
                ps[:, 0, :],
                lhsT=xT[:, i, t * 128:(t + 1) * 128],
                rhs=Wbf["v"][:, i, h * 512:(h + 1) * 512],
                start=(i == 0),
                stop=(i == NJ - 1),
            )

        def evict():
            nc.vector.tensor_tensor(
                out=Vno[:, t, h * 8:(h + 1) * 8, 0:DK],
                in0=ps[:, 0, :].rearrange("p (c d) -> p c d", c=8),
                in1=bvB[:, h * 512:(h + 1) * 512].rearrange("p (c d) -> p c d", c=8),
                op=ALU.add,
            )

        return [mm(i) for i in range(NJ)], evict

    def qk_chain(wname, bsb, dst, j):
        ps = psum_s.tile([128, 3, 512], F32, tag="s", name="psqk")

        def mm(i):
            return lambda: nc.tensor.matmul(
                ps[:, 0, :],
                lhsT=Wbf[wname][:, i, j * 128:(j + 1) * 128],
                rhs=xT[:, i, :],
                start=(i == 0),
                stop=(i == NJ - 1),
            )

        def evict():
            nc.scalar.activation(
                dst[:, j, :], ps[:, 0, :], AF.Identity, bias=bsb[:, j:j + 1]
            )

        return [mm(i) for i in range(NJ)], evict

    chains = [v_chain(0, 0), v_chain(0, 1)]
    for j in range(NJ):
        chains.append(qk_chain("q", bq_sb, QT, j))
        chains.append(qk_chain("k", bk_sb, KT, j))
    for t in range(1, T):
        for h in range(2):
            chains.append(v_chain(t, h))
    for a in range(0, len(chains), 2):
        mmsA, evA = chains[a]
        mmsB, evB = chains[a + 1]
        _interleave(mmsA, mmsB)
        evA()
        evB()

    # ---- stage 2: reorg into QhF/KhF (head data in partitions 0:64,
    # duplicated into 64:128 so two kc-chunks can run as concurrent
    # tile_position row-packed matmuls) ----
    # QhF[d', t, c*128+r] = QT[(c%2)*64+d', c//2, t*128+r]
    for t in range(T):
        for srcT, dsthF in ((QT, QhF), (KT, KhF)):
            for ce in range(2):
                src = srcT[ce * 64:(ce + 1) * 64, :, t * 128:(t + 1) * 128]
                dst = dsthF[0:64, t, :].rearrange("p (c r) -> p c r", c=16)[:, ce::2, :]
                nc.vector.tensor_copy(out=dst, in_=src)
        for dsthF in (QhF, KhF):
            nc.vector.tensor_copy(out=dsthF[64:128, t, :], in_=dsthF[0:64, t, :])

    # ---- stage 3: attention (software-pipelined scores/ctx interleave) ----
    if DEBUG:
        dbg_ctx = nc.dram_tensor("dbg_ctx", [4, DK + 1, 512], F32, kind="ExternalOutput")
        dbg_pr = nc.dram_tensor("dbg_pr", [128, 3, 512], F32, kind="ExternalOutput")
        dbg_s = nc.dram_tensor("dbg_s", [128, 3, 512], F32, kind="ExternalOutput")
        dbg_rsum = nc.dram_tensor("dbg_rsum", [4, 64, 512], F32, kind="ExternalOutput")
    for t in range(T):
        for qq in range(4):
            ctx = psum_ctx.tile([DK + 1, 512], F32, tag="ctx")
            pend_ctx = []  # ctx-matmul thunks of the previous group
            kc0 = 0
            for gi, gsz in enumerate(GROUPS):
                s = psum_s.tile([128, 3, 512], F32, tag="s", name="satt")

                def mk_s(u, kc, lo):
                    def f(s=s, u=u, kc=kc, lo=lo):
                        nc.tensor.matmul(
                            s[:, u, :],
                            lhsT=KhF[lo:lo + 64, t, kc * 128:(kc + 1) * 128],
                            rhs=QhF[lo:lo + 64, t, qq * 512:(qq + 1) * 512],
                            start=True,
                            stop=True,
                        )
                    return f

                # row-packed pairs: even kc on array rows 0-63, odd on 64-127,
                # emitted adjacently so they run concurrently.
                sco = []
                u = 0
                while u < gsz:
                    if u + 1 < gsz:
                        a = mk_s(u, kc0 + u, 0)
                        b = mk_s(u + 1, kc0 + u + 1, 64)
                        sco.append(lambda a=a, b=b: (a(), b()))
                        u += 2
                    else:
                        sco.append(mk_s(u, kc0 + u, 0))
                        u += 1
                _interleave(sco, pend_ctx)
                if DEBUG and t == 0 and qq == 0 and gi == 0:
                    sf = stage.tile([128, 3, 512], F32, tag="prf")
                    nc.vector.tensor_copy(out=sf[:], in_=s[:])
                    nc.scalar.dma_start(out=dbg_s[:], in_=sf[:])
                pr = stage.tile([128, 3, 512], BF16, tag="pr", bufs=3)
                nc.scalar.activation(
                    pr[:, 0:gsz, :], s[:, 0:gsz, :], AF.Exp, scale=0.125
                )
                if DEBUG and t == 0 and qq == 0 and gi == 0:
                    prf = stage.tile([128, 3, 512], F32, tag="prf")
                    nc.vector.tensor_copy(out=prf[:], in_=pr[:])
                    nc.scalar.dma_start(out=dbg_pr[:], in_=prf[:])
                pend_ctx = []
                for u in range(gsz):
                    kc = kc0 + u

                    def mk_c(u=u, kc=kc, pr=pr):
                        nc.tensor.matmul(
                            ctx[:],
                            lhsT=Vno[:, t, kc, :],
                            rhs=pr[:, u, :],
                            start=(kc == 0),
                            stop=(kc == 15),
                        )

                    pend_ctx.append(mk_c)
                kc0 += gsz
            for c in pend_ctx:
                c()
            if DEBUG and t == 0:
                cf = stage.tile([DK + 1, 512], F32, tag="ctxf")
                nc.vector.tensor_copy(out=cf[:], in_=ctx[:])
                nc.scalar.dma_start(out=dbg_ctx[qq, :, :], in_=cf[:])

            # normalize by softmax sums (ctx row 64) and reorg into cbT:
            # cbT[(c%2)*64+d', t, c//2, r] = ctx[d', (c-4qq)*128+r] / sums
            sums = stage.tile([1, 512], F32, tag="sums")
            nc.vector.tensor_copy(out=sums[:], in_=ctx[DK:DK + 1, :])
            rsum = stage.tile([1, 512], F32, tag="rsum")
            nc.vector.reciprocal_approx_fast(out=rsum[:], in_=sums[:])
            rsumB = stage.tile([64, 512], F32, tag="rsumB")
            nc.gpsimd.partition_broadcast(rsumB[:], rsum[:])
            if DEBUG and t == 0:
                nc.scalar.dma_start(out=dbg_rsum[qq, :, :], in_=rsumB[:])
            for ce in range(2):
                csrc = ctx[0:64, :].rearrange("p (c r) -> p c r", c=4)[:, ce::2, :]
                sc = rsumB[:].rearrange("p (c r) -> p c r", c=4)[:, ce::2, :]
                dst = cbT[ce * 64:(ce + 1) * 64, t, 2 * qq:2 * qq + 2, :]
                nc.vector.tensor_tensor(out=dst, in0=csrc, in1=sc, op=ALU.mult)

    # ---- stage 4: output projection (pairwise-interleaved chains) ----
    def o_chain(t, h):
        ps = psum_s.tile([128, 3, 512], F32, tag="s", name="pso")

        def mm(j):
            return lambda: nc.tensor.matmul(
                ps[:, 0, :],
                lhsT=cbT[:, t, j, :],
                rhs=Wbf["o"][:, j, h * 512:(h + 1) * 512],
                start=(j == 0),
                stop=(j == NJ - 1),
            )

        def evict():
            osb = stage.tile([128, 512], F32, tag="ostage")
            nc.vector.tensor_tensor(
                out=osb[:], in0=ps[:, 0, :], in1=boB[:, h * 512:(h + 1) * 512],
                op=ALU.add,
            )
            nc.scalar.dma_start(
                out=out_d[t * 128:(t + 1) * 128, h * 512:(h + 1) * 512], in_=osb[:]
            )

        return [mm(j) for j in range(NJ)], evict

    ochains = [o_chain(t, h) for t in range(T) for h in range(2)]
    for a in range(0, len(ochains), 2):
        mmsA, evA = ochains[a]
        mmsB, evB = ochains[a + 1]
        _interleave(mmsA, mmsB)
        evA()
        evB()

    if DEBUG:
        for nm, sb in (("dbg_xT", xT), ("dbg_QT", QT), ("dbg_KT", KT),
                       ("dbg_Vno", Vno), ("dbg_QhT", QhF), ("dbg_KhT", KhF),
                       ("dbg_cbT", cbT)):
            dd = nc.dram_tensor(nm, list(sb.shape), BF16, kind="ExternalOutput")
            nc.scalar.dma_start(out=dd[:], in_=sb[:])


_CACHE = {}


def build():
    if "nc" in _CACHE:
        return _CACHE["nc"]
    nc = bacc.Bacc(None, target_bir_lowering=False)
    with tile.TileContext(nc) as tc:
        import contextlib

        with contextlib.ExitStack() as ctx:
            pools = {
                "persist": ctx.enter_context(tc.tile_pool(name="persist", bufs=1)),
                "stage": ctx.enter_context(tc.tile_pool(name="stage", bufs=2)),
                "psum_s": ctx.enter_context(
                    tc.tile_pool(name="psum_s", bufs=2, space="PSUM")
                ),
                "psum_ctx": ctx.enter_context(
                    tc.tile_pool(name="psum_ctx", bufs=2, space="PSUM")
                ),
            }
            _emit(nc, tc, pools)
    nc.compile()
    _CACHE["nc"] = nc
    return nc


def kernel(x, Wq, bq, Wk, bk, Wv, bv, Wo, bo, _trace=False, _tmpdir=None):
    x = np.ascontiguousarray(np.asarray(x, dtype=np.float32))
    full = {
        "Wq": np.ascontiguousarray(np.asarray(Wq, np.float32)),
        "bq": np.ascontiguousarray(np.asarray(bq, np.float32)),
        "Wk": np.ascontiguousarray(np.asarray(Wk, np.float32)),
        "bk": np.ascontiguousarray(np.asarray(bk, np.float32)),
        "Wv": np.ascontiguousarray(np.asarray(Wv, np.float32)),
        "bv": np.ascontiguousarray(np.asarray(bv, np.float32)),
        "Wo": np.ascontiguousarray(np.asarray(Wo, np.float32)),
        "bo": np.ascontiguousarray(np.asarray(bo, np.float32)),
    }
    B, S, Dm = x.shape
    assert (B, S, Dm) == (2, 2048, 1024), (B, S, Dm)

    nc = build()
    in_maps = []
    for core in range(N_CORES):
        b, blk = core // 4, core % 4
        m = dict(full)
        m["x"] = np.ascontiguousarray(x[b, blk * ROWS:(blk + 1) * ROWS, :])
        in_maps.append(m)

    res = run_bass_kernel_spmd(
        nc,
        in_maps,
        core_ids=list(range(N_CORES)),
        trace=_trace,
        tmpdir=_tmpdir,
    )
    out = np.empty((B, S, Dm), np.float32)
    for core in range(N_CORES):
        b, blk = core // 4, core % 4
        out[b, blk * ROWS:(blk + 1) * ROWS, :] = res.results[core]["out"]
    if _trace:
        return out, res
    return out


# revision 15
# speedup vs baseline: 1.0443x; 1.0050x over previous
"""MultiHeadAttention (no-transpose head reshape) on 8 TRN2 NeuronCores.

The reference reshapes [B,S,D] -> [B,H,S',dk] WITHOUT transposing, so
"head h" of batch b is exactly rows [128h, 128h+128) of x viewed as 2048
pseudo-tokens of dim 64: pseudo-token (r, c) of head-block t is
x[t*128+r] features [c*64, c*64+64).  The whole problem is data-parallel
over the 32 (b,h) pairs: each of 8 cores owns 4 head-blocks (512 rows) of
one batch, no communication needed.

Internally pseudo-tokens are enumerated C-MAJOR (k'' = c*128 + r), which
is legal because softmax just sums over all keys (any consistent
permutation of keys works, and the query permutation is undone in the
final reorg).  With that ordering the "V with ones column" chunks are
natural slices of V, and every reorg copy moves contiguous 128-element
runs.

Per-core pipeline (bf16 matmuls, f32 accumulation):
  1. x -> bf16 -> xT via PE transposes (warms up the PE).
  2. V = x@Wv+bv straight into the ones-padded Vno layout;
     QT/KT = (x@W)^T with weight chunks stationary.
  3. QhT/KhT per-head [64,2048] via contiguous DVE copies.
  4. Attention per head: scoresT[k,q] = KhT^T@QhT (PSUM) -> exp on ACT
     (1/8 scale fused) -> probsT bf16 -> ctxT[65,q] += Vno^T@probsT,
     row 64 of ctxT accumulates softmax denominators (ones column).
  5. Normalize (DVE divide) + reorg into cbT (out-proj lhsT layout).
  6. out = cb@Wo + bo.

PE accumulation chains are emitted pairwise-interleaved so consecutive
matmuls hit different PSUM banks (fill overlaps drain).
"""

import sys

if "/opt/trn_rl_repo" not in sys.path:
    sys.path.insert(0, "/opt/trn_rl_repo")

import numpy as np

import concourse.bacc as bacc
import concourse.mybir as mybir
import concourse.tile as tile
from concourse.bass_utils import run_bass_kernel_spmd
from concourse.masks import make_identity

F32 = mybir.dt.float32
BF16 = mybir.dt.bfloat16
AF = mybir.ActivationFunctionType
ALU = mybir.AluOpType

N_CORES = 8
D = 1024
ROWS = 512          # rows of x per core
T = 4               # head-blocks (= heads) per core
NJ = 8              # 128-feature chunks of D
DK = 64
S2 = 2048           # pseudo-sequence length per head
GROUPS = (3, 3, 3, 3, 2, 2)   # k-chunks per exp group (sums to 16)
DEBUG = False


def _interleave(*seqs):
    """Round-robin the callables in seqs (lists of thunks), call in order."""
    n = max(len(s) for s in seqs)
    for u in range(n):
        for s in seqs:
            if u < len(s):
                s[u]()


def _emit(nc, tc, pools):
    persist = pools["persist"]
    stage = pools["stage"]
    psum_s = pools["psum_s"]      # tag "s": [128,3,512] f32, bufs=2 (6 banks)
    psum_ctx = pools["psum_ctx"]  # tag "ctx": [65,512] f32, bufs=2 (2 banks)

    x_d = nc.dram_tensor("x", [ROWS, D], F32, kind="ExternalInput")
    w_d = {}
    b_d = {}
    for w in ("q", "k", "v", "o"):
        w_d[w] = nc.dram_tensor(f"W{w}", [D, D], F32, kind="ExternalInput")
        b_d[w] = nc.dram_tensor(f"b{w}", [D], F32, kind="ExternalInput")
    out_d = nc.dram_tensor("out", [ROWS, D], F32, kind="ExternalOutput")

    # ---- persistent SBUF tensors ----
    xT = persist.tile([128, NJ, ROWS], BF16, name="xT")
    Wbf = {w: persist.tile([128, NJ, D], BF16, name=f"W{w}bf") for w in ("v", "q", "k", "o")}
    QT = persist.tile([128, NJ, ROWS], BF16, name="QT")
    KT = persist.tile([128, NJ, ROWS], BF16, name="KT")
    Vno = persist.tile([128, T, 16, DK + 1], BF16, name="Vno")
    QhF = persist.tile([128, T, S2], BF16, name="QhF")
    KhF = persist.tile([128, T, S2], BF16, name="KhF")
    cbT = persist.tile([128, T, NJ, 128], BF16, name="cbT")
    bq_row = persist.tile([1, D], F32, name="bq_row")
    bk_row = persist.tile([1, D], F32, name="bk_row")
    bv_row = persist.tile([1, D], F32, name="bv_row")
    bo_row = persist.tile([1, D], F32, name="bo_row")
    ones_row = persist.tile([1, 512], F32, name="ones_row")
    ident = persist.tile([128, 128], BF16, name="ident")

    make_identity(nc, ident[:])

    # pre-warm the exp table-set while ACT is otherwise idle
    dummy = persist.tile([1, 8], F32, name="dummy")
    nc.vector.memset(dummy[:], 0.0)
    nc.scalar.activation(dummy[:], dummy[:], AF.Exp, scale=1.0)

    # ---- biases (small, early); added via K=1 matmuls into the psum ----
    for row, w in ((bq_row, "q"), (bk_row, "k"), (bv_row, "v"), (bo_row, "o")):
        nc.sync.dma_start(out=row[:], in_=b_d[w].ap().unsqueeze(0))
    nc.vector.memset(ones_row[:], 1.0)

    # ones columns of Vno (overwritten below except column 64)
    nc.gpsimd.memset(Vno[:], 1.0)

    # ---- stage 0: x -> xT (PE transposes; also warms up the PE) ----
    for t in range(T):
        xs = stage.tile([128, D], F32, tag="xstage")
        nc.sync.dma_start(out=xs[:], in_=x_d[t * 128:(t + 1) * 128, :])
        xb = stage.tile([128, D], BF16, tag="xbf")
        nc.vector.tensor_copy(out=xb[:], in_=xs[:])
        for i in range(NJ):
            tp = psum_s.tile([128, 128], BF16, tag="s", name="tp")
            nc.tensor.transpose(tp[:], xb[:, i * 128:(i + 1) * 128], ident[:])
            nc.scalar.copy(out=xT[:, i, t * 128:(t + 1) * 128], in_=tp[:])

    # ---- weights: DMA + cast, in consumption order (Wo on gpsimd: late) ----
    for w in ("q", "k", "v", "o"):
        for i in range(NJ):
            ws = stage.tile([128, D], F32, tag="wstage")
            nc.sync.dma_start(out=ws[:], in_=w_d[w][i * 128:(i + 1) * 128, :])
            if w == "o":
                nc.gpsimd.tensor_copy(out=Wbf[w][:, i, :], in_=ws[:])
            else:
                nc.vector.tensor_copy(out=Wbf[w][:, i, :], in_=ws[:])

    # ---- stage 1: projections ----
    # Chains run in triples sharing one 3-bank psum tile (one accumulator
    # per bank), round-robin interleaved so consecutive PE matmuls always
    # hit different banks and the next triple overlaps this one's
    # evictions (tag "s" has 2 slots).
    def v_chain(t, h, ps, u):
        def mm(i):
            return lambda: nc.tensor.matmul(
                ps[:, u, :],
                lhsT=xT[:, i, t * 128:(t + 1) * 128],
                rhs=Wbf["v"][:, i, h * 512:(h + 1) * 512],
                start=(i == 0),
                stop=False,
            )

        def bias_mm():
            nc.tensor.matmul(
                ps[:, u, :],
                lhsT=ones_row[:, 0:128],
                rhs=bv_row[:, h * 512:(h + 1) * 512],
                start=False,
                stop=True,
            )

        def evict():
            nc.scalar.copy(
                out=Vno[:, t, h * 8:(h + 1) * 8, 0:DK],
                in_=ps[:, u, :].rearrange("p (c d) -> p c d", c=8),
            )

        return [mm(i) for i in range(NJ)] + [bias_mm], evict

    def qk_chain(wname, brow, dst, j, ps, u):
        def mm(i):
            return lambda: nc.tensor.matmul(
                ps[:, u, :],
                lhsT=Wbf[wname][:, i, j * 128:(j + 1) * 128],
                rhs=xT[:, i, :],
                start=(i == 0),
                stop=False,
            )

        def bias_mm():
            nc.tensor.matmul(
                ps[:, u, :],
                lhsT=brow[:, j * 128:(j + 1) * 128],
                rhs=ones_row[:],
                start=False,
                stop=True,
            )

        def evict():
            nc.scalar.copy(out=dst[:, j, :], in_=ps[:, u, :])

        return [mm(i) for i in range(NJ)] + [bias_mm], evict

    specs = []
    for j in range(NJ):
        specs.append(("q", j))
    for j in range(NJ):
        specs.append(("k", j))
    for t in range(T):
        for h in range(2):
            specs.append(("v", (t, h)))
    for a in range(0, len(specs), 3):
        trip = specs[a:a + 3]
        ps = psum_s.tile([128, 3, 512], F32, tag="s", name="pproj")
        built = []
        for u, (kind, arg) in enumerate(trip):
            if kind == "v":
                built.append(v_chain(arg[0], arg[1], ps, u))
            elif kind == "q":
                built.append(qk_chain("q", bq_row, QT, arg, ps, u))
            else:
                built.append(qk_chain("k", bk_row, KT, arg, ps, u))
        _interleave(*[b[0] for b in built])
        for b in built:
            b[1]()

    # ---- stage 2: reorg into QhF/KhF (head data in partitions 0:64,
    # duplicated into 64:128 so two kc-chunks can run as concurrent
    # tile_position row-packed matmuls) ----
    # QhF[d', t, c*128+r] = QT[(c%2)*64+d', c//2, t*128+r]
    for t in range(T):
        for srcT, dsthF in ((QT, QhF), (KT, KhF)):
            for ce in range(2):
                src = srcT[ce * 64:(ce + 1) * 64, :, t * 128:(t + 1) * 128]
                dst = dsthF[0:64, t, :].rearrange("p (c r) -> p c r", c=16)[:, ce::2, :]
                nc.vector.tensor_copy(out=dst, in_=src)
        for dsthF in (QhF, KhF):
            nc.vector.tensor_copy(out=dsthF[64:128, t, :], in_=dsthF[0:64, t, :])

    # ---- stage 3: attention (software-pipelined scores/ctx interleave) ----
    if DEBUG:
        dbg_ctx = nc.dram_tensor("dbg_ctx", [4, DK + 1, 512], F32, kind="ExternalOutput")
        dbg_pr = nc.dram_tensor("dbg_pr", [128, 3, 512], F32, kind="ExternalOutput")
        dbg_s = nc.dram_tensor("dbg_s", [128, 3, 512], F32, kind="ExternalOutput")
        dbg_rsum = nc.dram_tensor("dbg_rsum", [4, 64, 512], F32, kind="ExternalOutput")
    for t in range(T):
        for qq in range(4):
            ctx = psum_ctx.tile([DK + 1, 512], F32, tag="ctx")
            pend_ctx = []  # ctx-matmul thunks of the previous group
            kc0 = 0
            for gi, gsz in enumerate(GROUPS):
                s = psum_s.tile([128, 3, 512], F32, tag="s", name="satt")

                def mk_s(u, kc, lo):
                    def f(s=s, u=u, kc=kc, lo=lo):
                        nc.tensor.matmul(
                            s[:, u, :],
                            lhsT=KhF[lo:lo + 64, t, kc * 128:(kc + 1) * 128],
                            rhs=QhF[lo:lo + 64, t, qq * 512:(qq + 1) * 512],
                            start=True,
                            stop=True,
                        )
                    return f

                # row-packed pairs: even kc on array rows 0-63, odd on 64-127,
                # emitted adjacently so they run concurrently.
                sco = []
                u = 0
                while u < gsz:
                    if u + 1 < gsz:
                        a = mk_s(u, kc0 + u, 0)
                        b = mk_s(u + 1, kc0 + u + 1, 64)
                        sco.append(lambda a=a, b=b: (a(), b()))
                        u += 2
                    else:
                        sco.append(mk_s(u, kc0 + u, 0))
                        u += 1
                _interleave(sco, pend_ctx)
                if DEBUG and t == 0 and qq == 0 and gi == 0:
                    sf = stage.tile([128, 3, 512], F32, tag="prf")
                    nc.vector.tensor_copy(out=sf[:], in_=s[:])
                    nc.scalar.dma_start(out=dbg_s[:], in_=sf[:])
                pr = stage.tile([128, 3, 512], BF16, tag="pr", bufs=3)
                nc.scalar.activation(
                    pr[:, 0:gsz, :], s[:, 0:gsz, :], AF.Exp, scale=0.125
                )
                if DEBUG and t == 0 and qq == 0 and gi == 0:
                    prf = stage.tile([128, 3, 512], F32, tag="prf")
                    nc.vector.tensor_copy(out=prf[:], in_=pr[:])
                    nc.scalar.dma_start(out=dbg_pr[:], in_=prf[:])
                pend_ctx = []
                for u in range(gsz):
                    kc = kc0 + u

                    def mk_c(u=u, kc=kc, pr=pr):
                        nc.tensor.matmul(
                            ctx[:],
                            lhsT=Vno[:, t, kc, :],
                            rhs=pr[:, u, :],
                            start=(kc == 0),
                            stop=(kc == 15),
                        )

                    pend_ctx.append(mk_c)
                kc0 += gsz
            for c in pend_ctx:
                c()
            if DEBUG and t == 0:
                cf = stage.tile([DK + 1, 512], F32, tag="ctxf")
                nc.vector.tensor_copy(out=cf[:], in_=ctx[:])
                nc.scalar.dma_start(out=dbg_ctx[qq, :, :], in_=cf[:])

            # normalize by softmax sums (ctx row 64) and reorg into cbT:
            # cbT[(c%2)*64+d', t, c//2, r] = ctx[d', (c-4qq)*128+r] / sums
            sums = stage.tile([1, 512], F32, tag="sums")
            nc.vector.tensor_copy(out=sums[:], in_=ctx[DK:DK + 1, :])
            rsum = stage.tile([1, 512], F32, tag="rsum")
            nc.vector.reciprocal_approx_fast(out=rsum[:], in_=sums[:])
            rsumB = stage.tile([64, 512], F32, tag="rsumB")
            nc.gpsimd.partition_broadcast(rsumB[:], rsum[:])
            if DEBUG and t == 0:
                nc.scalar.dma_start(out=dbg_rsum[qq, :, :], in_=rsumB[:])
            for ce in range(2):
                csrc = ctx[0:64, :].rearrange("p (c r) -> p c r", c=4)[:, ce::2, :]
                sc = rsumB[:].rearrange("p (c r) -> p c r", c=4)[:, ce::2, :]
                dst = cbT[ce * 64:(ce + 1) * 64, t, 2 * qq:2 * qq + 2, :]
                nc.vector.tensor_tensor(out=dst, in0=csrc, in1=sc, op=ALU.mult)

        # output projection for this head, overlapping the next head's
        # ACT-bound attention (bias via K=1 matmul, eviction on DVE).
        po = psum_s.tile([128, 3, 512], F32, tag="s", name="po")

        def o_mms(h, u):
            def mm(j):
                return lambda: nc.tensor.matmul(
                    po[:, u, :],
                    lhsT=cbT[:, t, j, :],
                    rhs=Wbf["o"][:, j, h * 512:(h + 1) * 512],
                    start=(j == 0),
                    stop=False,
                )

            def bias_mm():
                nc.tensor.matmul(
                    po[:, u, :],
                    lhsT=ones_row[:, 0:128],
                    rhs=bo_row[:, h * 512:(h + 1) * 512],
                    start=False,
                    stop=True,
                )

            return [mm(j) for j in range(NJ)] + [bias_mm]

        _interleave(o_mms(0, 0), o_mms(1, 1))
        for h in range(2):
            osb = stage.tile([128, 512], F32, tag="ostage")
            nc.vector.tensor_copy(out=osb[:], in_=po[:, h, :])
            nc.sync.dma_start(
                out=out_d[t * 128:(t + 1) * 128, h * 512:(h + 1) * 512], in_=osb[:]
            )


    if DEBUG:
        for nm, sb in (("dbg_xT", xT), ("dbg_QT", QT), ("dbg_KT", KT),
                       ("dbg_Vno", Vno), ("dbg_QhT", QhF), ("dbg_KhT", KhF),
                       ("dbg_cbT", cbT)):
            dd = nc.dram_tensor(nm, list(sb.shape), BF16, kind="ExternalOutput")
            nc.scalar.dma_start(out=dd[:], in_=sb[:])


_CACHE = {}


def build():
    if "nc" in _CACHE:
        return _CACHE["nc"]
    nc = bacc.Bacc(None, target_bir_lowering=False)
    with tile.TileContext(nc) as tc:
        import contextlib

        with contextlib.ExitStack() as ctx:
            pools = {
                "persist": ctx.enter_context(tc.tile_pool(name="persist", bufs=1)),
                "stage": ctx.enter_context(tc.tile_pool(name="stage", bufs=2)),
                "psum_s": ctx.enter_context(
                    tc.tile_pool(name="psum_s", bufs=2, space="PSUM")
                ),
                "psum_ctx": ctx.enter_context(
                    tc.tile_pool(name="psum_ctx", bufs=2, space="PSUM")
                ),
            }
            _emit(nc, tc, pools)
    nc.compile()
    _CACHE["nc"] = nc
    return nc


def kernel(x, Wq, bq, Wk, bk, Wv, bv, Wo, bo, _trace=False, _tmpdir=None):
    x = np.ascontiguousarray(np.asarray(x, dtype=np.float32))
    full = {
        "Wq": np.ascontiguousarray(np.asarray(Wq, np.float32)),
        "bq": np.ascontiguousarray(np.asarray(bq, np.float32)),
        "Wk": np.ascontiguousarray(np.asarray(Wk, np.float32)),
        "bk": np.ascontiguousarray(np.asarray(bk, np.float32)),
        "Wv": np.ascontiguousarray(np.asarray(Wv, np.float32)),
        "bv": np.ascontiguousarray(np.asarray(bv, np.float32)),
        "Wo": np.ascontiguousarray(np.asarray(Wo, np.float32)),
        "bo": np.ascontiguousarray(np.asarray(bo, np.float32)),
    }
    B, S, Dm = x.shape
    assert (B, S, Dm) == (2, 2048, 1024), (B, S, Dm)

    nc = build()
    in_maps = []
    for core in range(N_CORES):
        b, blk = core // 4, core % 4
        m = dict(full)
        m["x"] = np.ascontiguousarray(x[b, blk * ROWS:(blk + 1) * ROWS, :])
        in_maps.append(m)

    res = run_bass_kernel_spmd(
        nc,
        in_maps,
        core_ids=list(range(N_CORES)),
        trace=_trace,
        tmpdir=_tmpdir,
    )
    out = np.empty((B, S, Dm), np.float32)
    for core in range(N_CORES):
        b, blk = core // 4, core % 4
        out[b, blk * ROWS:(blk + 1) * ROWS, :] = res.results[core]["out"]
    if _trace:
        return out, res
    return out


# revision 16
# speedup vs baseline: 1.1231x; 1.0755x over previous
"""MultiHeadAttention (no-transpose head reshape) on 8 TRN2 NeuronCores.

The reference reshapes [B,S,D] -> [B,H,S',dk] WITHOUT transposing, so
"head h" of batch b is exactly rows [128h, 128h+128) of x viewed as 2048
pseudo-tokens of dim 64: pseudo-token (r, c) of head-block t is
x[t*128+r] features [c*64, c*64+64).  The whole problem is data-parallel
over the 32 (b,h) pairs: each of 8 cores owns 4 head-blocks (512 rows) of
one batch, no communication needed.

Internally pseudo-tokens are enumerated C-MAJOR (k'' = c*128 + r), which
is legal because softmax just sums over all keys (any consistent
permutation of keys works, and the query permutation is undone in the
final reorg).  With that ordering the "V with ones column" chunks are
natural slices of V, and every reorg copy moves contiguous 128-element
runs.

Per-core pipeline (bf16 matmuls, f32 accumulation):
  1. x -> bf16 -> xT via PE transposes (warms up the PE).
  2. V = x@Wv+bv straight into the ones-padded Vno layout;
     QT/KT = (x@W)^T with weight chunks stationary.
  3. QhT/KhT per-head [64,2048] via contiguous DVE copies.
  4. Attention per head: scoresT[k,q] = KhT^T@QhT (PSUM) -> exp on ACT
     (1/8 scale fused) -> probsT bf16 -> ctxT[65,q] += Vno^T@probsT,
     row 64 of ctxT accumulates softmax denominators (ones column).
  5. Normalize (DVE divide) + reorg into cbT (out-proj lhsT layout).
  6. out = cb@Wo + bo.

PE accumulation chains are emitted pairwise-interleaved so consecutive
matmuls hit different PSUM banks (fill overlaps drain).
"""

import sys

if "/opt/trn_rl_repo" not in sys.path:
    sys.path.insert(0, "/opt/trn_rl_repo")

import numpy as np

import concourse.bacc as bacc
import concourse.mybir as mybir
import concourse.tile as tile
from concourse.bass_utils import run_bass_kernel_spmd
from concourse.masks import make_identity

F32 = mybir.dt.float32
BF16 = mybir.dt.bfloat16
AF = mybir.ActivationFunctionType
ALU = mybir.AluOpType

N_CORES = 8
D = 1024
ROWS = 512          # rows of x per core
T = 4               # head-blocks (= heads) per core
NJ = 8              # 128-feature chunks of D
DK = 64
S2 = 2048           # pseudo-sequence length per head
GROUPS = (3, 3, 3, 3, 2, 2)   # k-chunks per exp group (sums to 16)
DEBUG = False


def _interleave(*seqs):
    """Round-robin the callables in seqs (lists of thunks), call in order."""
    n = max(len(s) for s in seqs)
    for u in range(n):
        for s in seqs:
            if u < len(s):
                s[u]()


def _emit(nc, tc, pools):
    persist = pools["persist"]
    stage = pools["stage"]
    psum_s = pools["psum_s"]      # tag "s": [128,3,512] f32, bufs=2 (6 banks)
    psum_ctx = pools["psum_ctx"]  # tag "ctx": [65,512] f32, bufs=2 (2 banks)

    x_d = nc.dram_tensor("x", [ROWS, D], F32, kind="ExternalInput")
    w_d = {}
    b_d = {}
    for w in ("q", "k", "v", "o"):
        w_d[w] = nc.dram_tensor(f"W{w}", [D, D], F32, kind="ExternalInput")
        b_d[w] = nc.dram_tensor(f"b{w}", [D], F32, kind="ExternalInput")
    out_d = nc.dram_tensor("out", [ROWS, D], F32, kind="ExternalOutput")

    # ---- persistent SBUF tensors ----
    xT = persist.tile([128, NJ, ROWS], BF16, name="xT")
    Wbf = {w: persist.tile([128, NJ, D], BF16, name=f"W{w}bf") for w in ("v", "q", "k", "o")}
    QT = persist.tile([128, NJ, ROWS], BF16, name="QT")
    KT = persist.tile([128, NJ, ROWS], BF16, name="KT")
    Vno = persist.tile([128, T, 16, DK + 1], BF16, name="Vno")
    QhF = persist.tile([128, T, S2], BF16, name="QhF")
    KhF = persist.tile([128, T, S2], BF16, name="KhF")
    cbT = persist.tile([128, T, NJ, 128], BF16, name="cbT")
    bq_row = persist.tile([1, D], BF16, name="bq_row")
    bk_row = persist.tile([1, D], BF16, name="bk_row")
    bv_row = persist.tile([1, D], BF16, name="bv_row")
    bo_row = persist.tile([1, D], BF16, name="bo_row")
    ones_row = persist.tile([1, 512], BF16, name="ones_row")
    ident = persist.tile([128, 128], BF16, name="ident")

    make_identity(nc, ident[:])

    # pre-warm the exp table-set while ACT is otherwise idle
    dummy = persist.tile([1, 8], F32, name="dummy")
    nc.vector.memset(dummy[:], 0.0)
    nc.scalar.activation(dummy[:], dummy[:], AF.Exp, scale=1.0)

    # ---- biases (small, early); added via bf16 K=1 matmuls into the psum ----
    for row, w in ((bq_row, "q"), (bk_row, "k"), (bv_row, "v"), (bo_row, "o")):
        bstg = stage.tile([1, D], F32, tag="bstg")
        nc.sync.dma_start(out=bstg[:], in_=b_d[w].ap().unsqueeze(0))
        nc.vector.tensor_copy(out=row[:], in_=bstg[:])
    nc.vector.memset(ones_row[:], 1.0)

    # ones columns of Vno (overwritten below except column 64)
    nc.gpsimd.memset(Vno[:], 1.0)

    # ---- stage 0: x -> xT (PE transposes; also warms up the PE) ----
    for t in range(T):
        xs = stage.tile([128, D], F32, tag="xstage")
        nc.sync.dma_start(out=xs[:], in_=x_d[t * 128:(t + 1) * 128, :])
        xb = stage.tile([128, D], BF16, tag="xbf")
        nc.vector.tensor_copy(out=xb[:], in_=xs[:])
        for i in range(NJ):
            tp = psum_s.tile([128, 128], BF16, tag="s", name="tp")
            nc.tensor.transpose(tp[:], xb[:, i * 128:(i + 1) * 128], ident[:])
            nc.scalar.copy(out=xT[:, i, t * 128:(t + 1) * 128], in_=tp[:])

    # ---- weights: DMA + cast, in consumption order (Wo on gpsimd: late) ----
    for w in ("q", "k", "v", "o"):
        for i in range(NJ):
            ws = stage.tile([128, D], F32, tag="wstage")
            nc.sync.dma_start(out=ws[:], in_=w_d[w][i * 128:(i + 1) * 128, :])
            if w == "o":
                nc.gpsimd.tensor_copy(out=Wbf[w][:, i, :], in_=ws[:])
            else:
                nc.vector.tensor_copy(out=Wbf[w][:, i, :], in_=ws[:])

    # ---- stage 1: projections ----
    # Chains run in triples sharing one 3-bank psum tile (one accumulator
    # per bank), round-robin interleaved so consecutive PE matmuls always
    # hit different banks and the next triple overlaps this one's
    # evictions (tag "s" has 2 slots).
    def v_chain(t, h, ps, u):
        def mm(i):
            return lambda: nc.tensor.matmul(
                ps[:, u, :],
                lhsT=xT[:, i, t * 128:(t + 1) * 128],
                rhs=Wbf["v"][:, i, h * 512:(h + 1) * 512],
                start=(i == 0),
                stop=False,
            )

        def bias_mm():
            nc.tensor.matmul(
                ps[:, u, :],
                lhsT=ones_row[:, 0:128],
                rhs=bv_row[:, h * 512:(h + 1) * 512],
                start=False,
                stop=True,
            )

        def evict():
            nc.scalar.copy(
                out=Vno[:, t, h * 8:(h + 1) * 8, 0:DK],
                in_=ps[:, u, :].rearrange("p (c d) -> p c d", c=8),
            )

        return [mm(i) for i in range(NJ)] + [bias_mm], evict

    def qk_chain(wname, brow, dst, j, ps, u):
        def mm(i):
            return lambda: nc.tensor.matmul(
                ps[:, u, :],
                lhsT=Wbf[wname][:, i, j * 128:(j + 1) * 128],
                rhs=xT[:, i, :],
                start=(i == 0),
                stop=False,
            )

        def bias_mm():
            nc.tensor.matmul(
                ps[:, u, :],
                lhsT=brow[:, j * 128:(j + 1) * 128],
                rhs=ones_row[:],
                start=False,
                stop=True,
            )

        def evict():
            nc.scalar.copy(out=dst[:, j, :], in_=ps[:, u, :])

        return [mm(i) for i in range(NJ)] + [bias_mm], evict

    specs = []
    for j in range(NJ):
        specs.append(("q", j))
    for j in range(NJ):
        specs.append(("k", j))
    for t in range(T):
        for h in range(2):
            specs.append(("v", (t, h)))
    for a in range(0, len(specs), 3):
        trip = specs[a:a + 3]
        ps = psum_s.tile([128, 3, 512], F32, tag="s", name="pproj")
        built = []
        for u, (kind, arg) in enumerate(trip):
            if kind == "v":
                built.append(v_chain(arg[0], arg[1], ps, u))
            elif kind == "q":
                built.append(qk_chain("q", bq_row, QT, arg, ps, u))
            else:
                built.append(qk_chain("k", bk_row, KT, arg, ps, u))
        _interleave(*[b[0] for b in built])
        for b in built:
            b[1]()

    # ---- stage 2: reorg into QhF/KhF (head data in partitions 0:64,
    # duplicated into 64:128 so two kc-chunks can run as concurrent
    # tile_position row-packed matmuls) ----
    # QhF[d', t, c*128+r] = QT[(c%2)*64+d', c//2, t*128+r]
    for t in range(T):
        for srcT, dsthF in ((QT, QhF), (KT, KhF)):
            for ce in range(2):
                src = srcT[ce * 64:(ce + 1) * 64, :, t * 128:(t + 1) * 128]
                dst = dsthF[0:64, t, :].rearrange("p (c r) -> p c r", c=16)[:, ce::2, :]
                nc.vector.tensor_copy(out=dst, in_=src)
        for dsthF in (QhF, KhF):
            nc.vector.tensor_copy(out=dsthF[64:128, t, :], in_=dsthF[0:64, t, :])

    # ---- stage 3: attention (software-pipelined scores/ctx interleave) ----
    if DEBUG:
        dbg_ctx = nc.dram_tensor("dbg_ctx", [4, DK + 1, 512], F32, kind="ExternalOutput")
        dbg_pr = nc.dram_tensor("dbg_pr", [128, 3, 512], F32, kind="ExternalOutput")
        dbg_s = nc.dram_tensor("dbg_s", [128, 3, 512], F32, kind="ExternalOutput")
        dbg_rsum = nc.dram_tensor("dbg_rsum", [4, 64, 512], F32, kind="ExternalOutput")
    for t in range(T):
        for qq in range(4):
            ctx = psum_ctx.tile([DK + 1, 512], F32, tag="ctx")
            pend_ctx = []  # ctx-matmul thunks of the previous group
            kc0 = 0
            for gi, gsz in enumerate(GROUPS):
                s = psum_s.tile([128, 3, 512], F32, tag="s", name="satt")

                def mk_s(u, kc, lo):
                    def f(s=s, u=u, kc=kc, lo=lo):
                        nc.tensor.matmul(
                            s[:, u, :],
                            lhsT=KhF[lo:lo + 64, t, kc * 128:(kc + 1) * 128],
                            rhs=QhF[lo:lo + 64, t, qq * 512:(qq + 1) * 512],
                            start=True,
                            stop=True,
                        )
                    return f

                # row-packed pairs: even kc on array rows 0-63, odd on 64-127,
                # emitted adjacently so they run concurrently.
                sco = []
                u = 0
                while u < gsz:
                    if u + 1 < gsz:
                        a = mk_s(u, kc0 + u, 0)
                        b = mk_s(u + 1, kc0 + u + 1, 64)
                        sco.append(lambda a=a, b=b: (a(), b()))
                        u += 2
                    else:
                        sco.append(mk_s(u, kc0 + u, 0))
                        u += 1
                _interleave(sco, pend_ctx)
                if DEBUG and t == 0 and qq == 0 and gi == 0:
                    sf = stage.tile([128, 3, 512], F32, tag="prf")
                    nc.vector.tensor_copy(out=sf[:], in_=s[:])
                    nc.scalar.dma_start(out=dbg_s[:], in_=sf[:])
                pr = stage.tile([128, 3, 512], BF16, tag="pr", bufs=3)
                nc.scalar.activation(
                    pr[:, 0:gsz, :], s[:, 0:gsz, :], AF.Exp, scale=0.125
                )
                if DEBUG and t == 0 and qq == 0 and gi == 0:
                    prf = stage.tile([128, 3, 512], F32, tag="prf")
                    nc.vector.tensor_copy(out=prf[:], in_=pr[:])
                    nc.scalar.dma_start(out=dbg_pr[:], in_=prf[:])
                pend_ctx = []
                for u in range(gsz):
                    kc = kc0 + u

                    def mk_c(u=u, kc=kc, pr=pr):
                        nc.tensor.matmul(
                            ctx[:],
                            lhsT=Vno[:, t, kc, :],
                            rhs=pr[:, u, :],
                            start=(kc == 0),
                            stop=(kc == 15),
                        )

                    pend_ctx.append(mk_c)
                kc0 += gsz
            for c in pend_ctx:
                c()
            if DEBUG and t == 0:
                cf = stage.tile([DK + 1, 512], F32, tag="ctxf")
                nc.vector.tensor_copy(out=cf[:], in_=ctx[:])
                nc.scalar.dma_start(out=dbg_ctx[qq, :, :], in_=cf[:])

            # normalize by softmax sums (ctx row 64) and reorg into cbT:
            # cbT[(c%2)*64+d', t, c//2, r] = ctx[d', (c-4qq)*128+r] / sums
            sums = stage.tile([1, 512], F32, tag="sums")
            nc.vector.tensor_copy(out=sums[:], in_=ctx[DK:DK + 1, :])
            rsum = stage.tile([1, 512], F32, tag="rsum")
            nc.vector.reciprocal_approx_fast(out=rsum[:], in_=sums[:])
            rsumB = stage.tile([64, 512], F32, tag="rsumB")
            nc.gpsimd.partition_broadcast(rsumB[:], rsum[:])
            if DEBUG and t == 0:
                nc.scalar.dma_start(out=dbg_rsum[qq, :, :], in_=rsumB[:])
            for ce in range(2):
                csrc = ctx[0:64, :].rearrange("p (c r) -> p c r", c=4)[:, ce::2, :]
                sc = rsumB[:].rearrange("p (c r) -> p c r", c=4)[:, ce::2, :]
                dst = cbT[ce * 64:(ce + 1) * 64, t, 2 * qq:2 * qq + 2, :]
                nc.vector.tensor_tensor(out=dst, in0=csrc, in1=sc, op=ALU.mult)

        # output projection for this head, overlapping the next head's
        # ACT-bound attention (bias via K=1 matmul, eviction on DVE).
        po = psum_s.tile([128, 3, 512], F32, tag="s", name="po")

        def o_mms(h, u):
            def mm(j):
                return lambda: nc.tensor.matmul(
                    po[:, u, :],
                    lhsT=cbT[:, t, j, :],
                    rhs=Wbf["o"][:, j, h * 512:(h + 1) * 512],
                    start=(j == 0),
                    stop=False,
                )

            def bias_mm():
                nc.tensor.matmul(
                    po[:, u, :],
                    lhsT=ones_row[:, 0:128],
                    rhs=bo_row[:, h * 512:(h + 1) * 512],
                    start=False,
                    stop=True,
                )

            return [mm(j) for j in range(NJ)] + [bias_mm]

        _interleave(o_mms(0, 0), o_mms(1, 1))
        for h in range(2):
            osb = stage.tile([128, 512], F32, tag="ostage")
            nc.vector.tensor_copy(out=osb[:], in_=po[:, h, :])
            nc.sync.dma_start(
                out=out_d[t * 128:(t + 1) * 128, h * 512:(h + 1) * 512], in_=osb[:]
            )


    if DEBUG:
        for nm, sb in (("dbg_xT", xT), ("dbg_QT", QT), ("dbg_KT", KT),
                       ("dbg_Vno", Vno), ("dbg_QhT", QhF), ("dbg_KhT", KhF),
                       ("dbg_cbT", cbT)):
            dd = nc.dram_tensor(nm, list(sb.shape), BF16, kind="ExternalOutput")
            nc.scalar.dma_start(out=dd[:], in_=sb[:])


_CACHE = {}


def build():
    if "nc" in _CACHE:
        return _CACHE["nc"]
    nc = bacc.Bacc(None, target_bir_lowering=False)
    with tile.TileContext(nc) as tc:
        import contextlib

        with contextlib.ExitStack() as ctx:
            pools = {
                "persist": ctx.enter_context(tc.tile_pool(name="persist", bufs=1)),
                "stage": ctx.enter_context(tc.tile_pool(name="stage", bufs=2)),
                "psum_s": ctx.enter_context(
                    tc.tile_pool(name="psum_s", bufs=2, space="PSUM")
                ),
                "psum_ctx": ctx.enter_context(
                    tc.tile_pool(name="psum_ctx", bufs=2, space="PSUM")
                ),
            }
            _emit(nc, tc, pools)
    nc.compile()
    _CACHE["nc"] = nc
    return nc


def kernel(x, Wq, bq, Wk, bk, Wv, bv, Wo, bo, _trace=False, _tmpdir=None):
    x = np.ascontiguousarray(np.asarray(x, dtype=np.float32))
    full = {
        "Wq": np.ascontiguousarray(np.asarray(Wq, np.float32)),
        "bq": np.ascontiguousarray(np.asarray(bq, np.float32)),
        "Wk": np.ascontiguousarray(np.asarray(Wk, np.float32)),
        "bk": np.ascontiguousarray(np.asarray(bk, np.float32)),
        "Wv": np.ascontiguousarray(np.asarray(Wv, np.float32)),
        "bv": np.ascontiguousarray(np.asarray(bv, np.float32)),
        "Wo": np.ascontiguousarray(np.asarray(Wo, np.float32)),
        "bo": np.ascontiguousarray(np.asarray(bo, np.float32)),
    }
    B, S, Dm = x.shape
    assert (B, S, Dm) == (2, 2048, 1024), (B, S, Dm)

    nc = build()
    in_maps = []
    for core in range(N_CORES):
        b, blk = core // 4, core % 4
        m = dict(full)
        m["x"] = np.ascontiguousarray(x[b, blk * ROWS:(blk + 1) * ROWS, :])
        in_maps.append(m)

    res = run_bass_kernel_spmd(
        nc,
        in_maps,
        core_ids=list(range(N_CORES)),
        trace=_trace,
        tmpdir=_tmpdir,
    )
    out = np.empty((B, S, Dm), np.float32)
    for core in range(N_CORES):
        b, blk = core // 4, core % 4
        out[b, blk * ROWS:(blk + 1) * ROWS, :] = res.results[core]["out"]
    if _trace:
        return out, res
    return out
